# revision 1
# baseline (speedup 1.0000x reference)
"""Single-head attention kernel for Trainium2, SPMD over 8 NeuronCores.

Problem: out = softmax((q@Wq+bq) @ (k@Wk+bk)^T / sqrt(768)) @ (v@Wv+bv)
Shapes: q,k,v [8, 2048, 768] fp32; W* [768, 64]; b* [64].

Strategy: data-parallel over batch (1 batch per core). Host transposes
q/k/v to [768, 2048] and casts to fp16 (layout prep only, no FLOPs on
host). On device, per core:
  - inputs stream in s-chunks so projections (and the softmax loop)
    start long before the full 9.4 MB has landed.
  - projections: qiT/kiT/viT = W.T @ xT accumulated fp32 in PSUM over
    6 e-chunks of 128. W is fed duplicated [768, 128] so both partition
    halves of the [128, 2048] projection output hold identical copies.
  - viT is transposed back to vi [t, h] tiles with PE transpose-mode
    matmuls against a host-fed identity, packed as [vi | ones] blocks.
  - per t-block (16 x 128 keys): scores^T [t, s] = kiT-block.T @ qiT
    (K=64 contraction) into PSUM, exp on ScalarE with the 1/sqrt(768)
    scale fused into the activation (scaled scores are N(0, 1/12), so
    a stable-softmax max-subtraction is unnecessary), then the output
    matmuls for that block accumulate into a persistent PSUM region.
    lhsT = [vi_block | ones] makes PSUM rows 64-127 accumulate the
    softmax denominator replicated across partitions.
  - normalize with reciprocal_approx_fast + multiply, DMA out^T fp32.
"""

import numpy as np
from contextlib import ExitStack

import concourse.bass as bass
import concourse.mybir as mybir
import concourse.tile as tile
from concourse import bacc
from concourse.bass_utils import run_bass_kernel_spmd

E = 768  # n_embd
H = 64  # head size
S = 2048  # sequence length
B = 8  # batch == n_cores
EC = E // 128  # e chunks
TB = S // 128  # t blocks
INV_SQRT_C = float(1.0 / np.sqrt(np.float32(E)))

F16 = mybir.dt.float16
F32 = mybir.dt.float32

_CACHE = {}


def build_program():
    nc = bacc.Bacc(
        "TRN2",
        target_bir_lowering=False,
        debug=False,
        enable_asserts=False,
        num_devices=B,
    )

    qT_d = nc.dram_tensor("qT", [E, S], F16, kind="ExternalInput")
    kT_d = nc.dram_tensor("kT", [E, S], F16, kind="ExternalInput")
    vT_d = nc.dram_tensor("vT", [E, S], F16, kind="ExternalInput")
    wq_d = nc.dram_tensor("wq", [E, 128], F16, kind="ExternalInput")
    wk_d = nc.dram_tensor("wk", [E, 128], F16, kind="ExternalInput")
    wv_d = nc.dram_tensor("wv", [E, 128], F16, kind="ExternalInput")
    bq_d = nc.dram_tensor("bq", [128, 1], F32, kind="ExternalInput")
    bk_d = nc.dram_tensor("bk", [128, 1], F32, kind="ExternalInput")
    bv_d = nc.dram_tensor("bv", [128, 1], F32, kind="ExternalInput")
    id_d = nc.dram_tensor("ident", [H, H], F16, kind="ExternalInput")
    outT_d = nc.dram_tensor("outT", [H, S], F32, kind="ExternalOutput")

    with tile.TileContext(nc) as tc, ExitStack() as ctx:
        const = ctx.enter_context(tc.tile_pool(name="const", bufs=1))
        xin = ctx.enter_context(tc.tile_pool(name="xin", bufs=1))
        acts = ctx.enter_context(tc.tile_pool(name="acts", bufs=1))
        attp = ctx.enter_context(tc.tile_pool(name="attp", bufs=16))

        # ---- constants ----
        wq_t = const.tile([128, EC * 128], F16, tag="wq")
        wk_t = const.tile([128, EC * 128], F16, tag="wk")
        wv_t = const.tile([128, EC * 128], F16, tag="wv")
        bq_t = const.tile([128, 1], F32, tag="bq")
        bk_t = const.tile([128, 1], F32, tag="bk")
        bv_t = const.tile([128, 1], F32, tag="bv")
        id_t = const.tile([H, H], F16, tag="ident")
        warm = const.tile([128, 8], F32, tag="warm")
        for w_t, w_d in ((wq_t, wq_d), (wk_t, wk_d), (wv_t, wv_d)):
            nc.sync.dma_start(
                w_t[:].rearrange("p (c m) -> p c m", c=EC),
                w_d.rearrange("(c p) m -> p c m", p=128),
            )
        for b_t, b_d in ((bq_t, bq_d), (bk_t, bk_d), (bv_t, bv_d)):
            nc.sync.dma_start(b_t[:], b_d[:])
        nc.sync.dma_start(id_t[:], id_d[:])

        # prefetch the exp table set on ScalarE while DMAs run
        nc.vector.memset(warm[:], 0.0)
        nc.scalar.activation(
            warm[:], warm[:], mybir.ActivationFunctionType.Exp, scale=1.0
        )

        # ---- streamed input loads ----
        # q first (every score tile needs all of qiT), k in quarters
        # (kiT t-block tb unblocks at quarter tb//4), v late (only the
        # output matmuls need vi).
        q_in = xin.tile([128, EC * S], F16, tag="q_in")
        k_in = xin.tile([128, EC * S], F16, tag="k_in")
        v_in = xin.tile([128, EC * S], F16, tag="v_in")

        def load_schunk(x_t, x_d, ch, w):
            # s-columns [ch*w, (ch+1)*w) of all 6 e-chunks
            dst = x_t[:].rearrange("p (c s) -> p c s", s=S)[
                :, :, ch * w : (ch + 1) * w
            ]
            src = x_d.rearrange("(c p) s -> p c s", p=128)[
                :, :, ch * w : (ch + 1) * w
            ]
            nc.sync.dma_start(dst, src)

        load_schunk(q_in, qT_d, 0, 1024)
        load_schunk(k_in, kT_d, 0, 1024)
        load_schunk(k_in, kT_d, 1, 1024)
        load_schunk(q_in, qT_d, 1, 1024)
        load_schunk(v_in, vT_d, 0, 1024)
        load_schunk(v_in, vT_d, 1, 1024)

        qiT = acts.tile([128, S], F16, tag="qiT")
        kiT = acts.tile([128, S], F16, tag="kiT")
        viT = acts.tile([128, S], F16, tag="viT")
        vaug = acts.tile([128, TB * 128], F16, tag="vaug")
        out_sb = acts.tile([H, S], F32, tag="out_sb")
        recip = acts.tile([H, S], F32, tag="recip")

        nc.vector.memset(vaug[:], 1.0)

        with tc.tile_pool(name="ps", bufs=2, space="PSUM") as ps, tc.tile_pool(
            name="op", bufs=1, space="PSUM"
        ) as op:

            def proj_half(x_in, w_t, b_t, dst, h):
                # one 1024-wide s-half of a projection, weight-major: each
                # W e-chunk is loaded once and runs 2 consecutive matmuls,
                # keeping the PE array streaming instead of reloading
                # weights before every matmul.
                pj = ps.tile([128, 1024], F32, tag="ps")
                for c in range(EC):
                    for j in range(2):
                        nc.tensor.matmul(
                            pj[:, j * 512 : (j + 1) * 512],
                            lhsT=w_t[:, c * 128 : (c + 1) * 128],
                            rhs=x_in[
                                :,
                                c * S + h * 1024 + j * 512 : c * S
                                + h * 1024
                                + (j + 1) * 512,
                            ],
                            start=(c == 0),
                            stop=(c == EC - 1),
                        )
                nc.vector.tensor_scalar_add(
                    dst[:, h * 1024 : (h + 1) * 1024], pj[:], b_t[:]
                )

            def score_half(attT, tb, h):
                sc = ps.tile([128, 1024], F32, tag="ps")
                for j in range(2):
                    nc.tensor.matmul(
                        sc[:, j * 512 : (j + 1) * 512],
                        lhsT=kiT[0:H, tb * 128 : (tb + 1) * 128],
                        rhs=qiT[0:H, h * 1024 + j * 512 : h * 1024 + (j + 1) * 512],
                        start=True,
                        stop=True,
                    )
                nc.scalar.activation(
                    attT[:, h * 1024 : (h + 1) * 1024],
                    sc[:],
                    mybir.ActivationFunctionType.Exp,
                    scale=INV_SQRT_C,
                )

            # ---- pass A: h0 scores+exp, with remaining projections woven
            # into the PE stream at the points their DMAs have landed ----
            attTs = [
                attp.tile([128, S], F16, tag="attT", name=f"attT{i}")
                for i in range(TB)
            ]
            po_t = op.tile([128, S], F32, tag="op")

            proj_half(q_in, wq_t, bq_t, qiT, 0)
            proj_half(k_in, wk_t, bk_t, kiT, 0)
            for tb in range(8):
                score_half(attTs[tb], tb, 0)
            proj_half(k_in, wk_t, bk_t, kiT, 1)
            for tb in range(8, TB):
                score_half(attTs[tb], tb, 0)
            proj_half(q_in, wq_t, bq_t, qiT, 1)
            proj_half(v_in, wv_t, bv_t, viT, 0)
            proj_half(v_in, wv_t, bv_t, viT, 1)

            # viT [64, 2048] -> vi blocks [128, 64] into vaug via PE transpose
            for g in range(2):
                tr = ps.tile([128, 512], F16, tag="ps")
                for i in range(8):
                    tb = g * 8 + i
                    nc.tensor.transpose(
                        tr[:, i * 64 : (i + 1) * 64],
                        viT[0:H, tb * 128 : (tb + 1) * 128],
                        id_t[:],
                    )
                dst_ap = vaug[:, g * 1024 : (g + 1) * 1024].rearrange(
                    "p (t c) -> p t c", c=128
                )[:, :, 0:H]
                src_ap = tr[:].rearrange("p (t c) -> p t c", c=H)
                nc.vector.tensor_copy(dst_ap, src_ap)

            # ---- pass B: h1 scores+exp plus output accumulation ----
            for tb in range(TB):
                attT = attTs[tb]
                score_half(attT, tb, 1)
                for j in range(4):
                    nc.tensor.matmul(
                        po_t[:, j * 512 : (j + 1) * 512],
                        lhsT=vaug[:, tb * 128 : (tb + 1) * 128],
                        rhs=attT[:, j * 512 : (j + 1) * 512],
                        start=(tb == 0),
                        stop=(tb == TB - 1),
                    )

            # normalize: rows 0-63 = unnormalized out^T, rows 64-127 = denom
            # (bounce denom to a base-0 SBUF tile first: custom-DVE ops do not
            # honor a nonzero base partition on HW)
            dsb = acts.tile([H, S], F32, tag="dsb")
            nc.vector.tensor_copy(dsb[:], po_t[H:128, :])
            nc.vector.reciprocal_approx_fast(recip[:], dsb[:])
            nc.vector.tensor_tensor(
                out_sb[:], po_t[0:H, :], recip[:], op=mybir.AluOpType.mult
            )
            nc.sync.dma_start(outT_d[:], out_sb[:])

    nc.compile()
    return nc


def _prep_inputs(q, k, v, Wq, bq, Wk, bk, Wv, bv):
    """Host-side layout prep: per-batch transpose + fp16 cast."""
    w2 = {}
    for name, W in (("wq", Wq), ("wk", Wk), ("wv", Wv)):
        w2[name] = np.ascontiguousarray(
            np.concatenate([W, W], axis=1), dtype=np.float16
        )
    b2 = {}
    for name, b in (("bq", bq), ("bk", bk), ("bv", bv)):
        b2[name] = np.ascontiguousarray(
            np.tile(np.asarray(b, dtype=np.float32).reshape(H, 1), (2, 1))
        )
    ident = np.eye(H, dtype=np.float16)
    in_maps = []
    for i in range(B):
        m = {
            "qT": np.ascontiguousarray(q[i].T, dtype=np.float16),
            "kT": np.ascontiguousarray(k[i].T, dtype=np.float16),
            "vT": np.ascontiguousarray(v[i].T, dtype=np.float16),
            "ident": ident,
        }
        m.update(w2)
        m.update(b2)
        in_maps.append(m)
    return in_maps


def run(trace=False, **inputs):
    """Build (cached), run on 8 cores, gather. Returns (out, BassKernelResults)."""
    if "nc" not in _CACHE:
        _CACHE["nc"] = build_program()
    nc = _CACHE["nc"]
    in_maps = _prep_inputs(**{k2: np.asarray(v2) for k2, v2 in inputs.items()})
    res = run_bass_kernel_spmd(nc, in_maps, list(range(B)), trace=trace)
    out = np.stack([np.ascontiguousarray(res.results[i]["outT"].T) for i in range(B)])
    return out.astype(np.float32), res


def kernel(**inputs) -> np.ndarray:
    out, _ = run(trace=False, **inputs)
    return out



# revision 2
# speedup vs baseline: 1.0702x; 1.0702x over previous
"""Single-head attention kernel for Trainium2, SPMD over 8 NeuronCores.

Problem: out = softmax((q@Wq+bq) @ (k@Wk+bk)^T / sqrt(768)) @ (v@Wv+bv)
Shapes: q,k,v [8, 2048, 768] fp32; W* [768, 64]; b* [64].

Strategy (v2): data-parallel over batch (1 batch per core). Host transposes
q/k/v to [768, 2048] fp16 (layout prep only). Per core the kernel is a
software-pipelined stream bounded by the ScalarE exp wall (4.2M exps ~= 30us):

  - DMA on two HWDGE queues: the Scalar queue carries the critical gate
    (wq/wk, q cols 0:1024, k cols 0:512) so scores start ASAP; the Sync
    queue carries the rest (k, v, q tail) ordered by need-time.
  - projections: x@W via W.T-as-lhsT accumulated over 6 e-chunks of 128.
    wq/wk are fed duplicated [768,128] so qiT/kiT hold 2 identical copies
    across the 128 partitions (feeds both row-tile groups).
  - scores: processed in (pair, j) units: pair p = t-blocks (2p, 2p+1),
    j = 512 q-columns.  Two K=64 matmuls run CONCURRENTLY in PE row
    groups 0-63 / 64-127 (tile_position row tiling), filling the two
    halves of one [128,1024] PSUM tile; a single FD-1024 exp on ScalarE
    (scale 1/sqrt(768) fused; scaled scores are N(0,1/12) so no
    max-subtraction) writes both t-blocks' att into one att slice.
  - v: viT = Wv.T @ vT (plain proj), then one batched DMA-xbar transpose
    per 1024-column half into a contiguous scratch, DVE-copied into
    vaug = [vi | ones] blocks (ones columns make PSUM rows 64-127 of the
    output accumulate the softmax denominator for free).
  - output: per (t-block, j) matmul accumulating into a persistent
    [128,2048] PSUM region, interleaved into the score stream with a lag
    so the in-order PE queue never blocks on late operands.
  - normalize per 512-column chunk as soon as its accumulation stops:
    copy denom, reciprocal_approx_fast, multiply, DMA out -> the tail
    after the last exp is ~2.5us instead of ~20us.
"""

import numpy as np
from contextlib import ExitStack

import concourse.bass as bass
import concourse.mybir as mybir
import concourse.tile as tile
from concourse import bacc
from concourse.bass_utils import run_bass_kernel_spmd

E = 768  # n_embd
H = 64  # head size
S = 2048  # sequence length
B = 8  # batch == n_cores
EC = E // 128  # e chunks
TB = S // 128  # t blocks
NP = TB // 2  # t-block pairs
INV_SQRT_C = float(1.0 / np.sqrt(np.float32(E)))

F16 = mybir.dt.float16
F32 = mybir.dt.float32

_CACHE = {}


def build_program():
    nc = bacc.Bacc(
        "TRN2",
        target_bir_lowering=False,
        debug=False,
        enable_asserts=False,
        num_devices=B,
    )

    qT_d = nc.dram_tensor("qT", [E, S], F16, kind="ExternalInput")
    kT_d = nc.dram_tensor("kT", [E, S], F16, kind="ExternalInput")
    vT_d = nc.dram_tensor("vT", [E, S], F16, kind="ExternalInput")
    wq_d = nc.dram_tensor("wq", [E, 128], F16, kind="ExternalInput")
    wk_d = nc.dram_tensor("wk", [E, 128], F16, kind="ExternalInput")
    wv_d = nc.dram_tensor("wv", [E, H], F16, kind="ExternalInput")
    bq_d = nc.dram_tensor("bq", [128, 1], F32, kind="ExternalInput")
    bk_d = nc.dram_tensor("bk", [128, 1], F32, kind="ExternalInput")
    bv_d = nc.dram_tensor("bv", [H, 1], F32, kind="ExternalInput")
    outT_d = nc.dram_tensor("outT", [H, S], F32, kind="ExternalOutput")

    with tile.TileContext(nc) as tc, ExitStack() as ctx:
        const = ctx.enter_context(tc.tile_pool(name="const", bufs=1))
        xin = ctx.enter_context(tc.tile_pool(name="xin", bufs=1))
        acts = ctx.enter_context(tc.tile_pool(name="acts", bufs=1))

        wq_t = const.tile([128, EC * 128], F16, tag="wq")
        wk_t = const.tile([128, EC * 128], F16, tag="wk")
        wv_t = const.tile([128, EC * H], F16, tag="wv")
        bq_t = const.tile([128, 1], F32, tag="bq")
        bk_t = const.tile([128, 1], F32, tag="bk")
        bv_t = const.tile([H, 1], F32, tag="bv")
        warm = const.tile([128, 8], F32, tag="warm")

        q_in = xin.tile([128, EC * S], F16, tag="q_in")
        k_in = xin.tile([128, EC * S], F16, tag="k_in")
        v_in = xin.tile([128, EC * S], F16, tag="v_in")

        qiT = acts.tile([128, S], F16, tag="qiT")
        kiT = acts.tile([128, S], F16, tag="kiT")
        viT = acts.tile([H, S], F16, tag="viT")
        vtr = acts.tile([128, S // 2], F16, tag="vtr")  # [128, 8, 64] x2 halves
        vaug = acts.tile([128, TB * 128], F16, tag="vaug")
        att = acts.tile([128, NP * 4096], F16, tag="att")
        dsb = acts.tile([H, S], F32, tag="dsb")
        rec = acts.tile([H, S], F32, tag="rec")
        out_sb = acts.tile([H, S], F32, tag="out_sb")

        def load_w(eng, w_t, w_d, m):
            eng.dma_start(
                w_t[:].rearrange("p (c m) -> p c m", c=EC),
                w_d.rearrange("(c p) m -> p c m", p=128),
            )

        def load_cols(eng, x_t, x_d, c0, w):
            # s-columns [c0, c0+w) of all 6 e-chunks
            dst = x_t[:].rearrange("p (c s) -> p c s", s=S)[:, :, c0 : c0 + w]
            src = x_d.rearrange("(c p) s -> p c s", p=128)[:, :, c0 : c0 + w]
            eng.dma_start(dst, src)

        # ---- Scalar HWDGE queue: the critical gate ----
        load_w(nc.scalar, wq_t, wq_d, 128)
        load_w(nc.scalar, wk_t, wk_d, 128)
        nc.scalar.dma_start(bq_t[:], bq_d[:])
        nc.scalar.dma_start(bk_t[:], bk_d[:])
        # warm the exp table set while the input DMAs stream
        nc.vector.memset(warm[:], 0.0)
        nc.scalar.activation(
            warm[:], warm[:], mybir.ActivationFunctionType.Exp, scale=1.0
        )
        load_cols(nc.scalar, q_in, qT_d, 0, 512)
        load_cols(nc.scalar, k_in, kT_d, 0, 512)
        load_cols(nc.scalar, q_in, qT_d, 512, 512)

        # ---- Sync HWDGE queue: the rest, in need-time order ----
        load_cols(nc.sync, k_in, kT_d, 512, 512)
        load_cols(nc.sync, k_in, kT_d, 1024, 512)
        load_cols(nc.sync, k_in, kT_d, 1536, 512)
        load_w(nc.sync, wv_t, wv_d, H)
        nc.sync.dma_start(bv_t[:], bv_d[:])
        load_cols(nc.sync, v_in, vT_d, 0, 1024)
        load_cols(nc.sync, q_in, qT_d, 1024, 512)
        load_cols(nc.sync, v_in, vT_d, 1024, 1024)
        load_cols(nc.sync, q_in, qT_d, 1536, 512)

        nc.vector.memset(vaug[:], 1.0)

        with tc.tile_pool(name="ps", bufs=2, space="PSUM") as ps, tc.tile_pool(
            name="op", bufs=1, space="PSUM"
        ) as op:
            po = op.tile([128, S], F32, tag="po")
            out_emitted = [0, 0, 0, 0]

            def proj_chunk(x_in, w_t, b_t, dst, c):
                # 512 s-columns of a q/k projection (dup'd weights)
                pj = ps.tile([128, 1024], F32, tag="ps")
                for e in range(EC):
                    nc.tensor.matmul(
                        pj[:, 0:512],
                        lhsT=w_t[:, e * 128 : (e + 1) * 128],
                        rhs=x_in[:, e * S + c * 512 : e * S + (c + 1) * 512],
                        start=(e == 0),
                        stop=(e == EC - 1),
                    )
                nc.vector.tensor_scalar_add(
                    dst[:, c * 512 : (c + 1) * 512], pj[:, 0:512], b_t[:]
                )

            def proj_v(h):
                # 1024 s-columns of the v projection (single-width weights)
                pj = ps.tile([128, 1024], F32, tag="ps")
                for e in range(EC):
                    for n in range(2):
                        nc.tensor.matmul(
                            pj[0:H, n * 512 : (n + 1) * 512],
                            lhsT=wv_t[:, e * H : (e + 1) * H],
                            rhs=v_in[
                                :,
                                e * S + h * 1024 + n * 512 : e * S
                                + h * 1024
                                + (n + 1) * 512,
                            ],
                            start=(e == 0),
                            stop=(e == EC - 1),
                        )
                nc.vector.tensor_scalar_add(
                    viT[:, h * 1024 : (h + 1) * 1024], pj[0:H, :], bv_t[:]
                )
                # viT[64, 1024] -> vtr [128, 8, 64] via DMA xbar transpose
                # (contiguous dst; strided dst is a HW bug), then DVE-copy
                # into the [vi | ones] vaug blocks.
                vtr_h = vtr[:, h * 512 : (h + 1) * 512].rearrange(
                    "p (t k) -> p t k", k=H
                )
                nc.sync.dma_start_transpose(
                    vtr_h, viT[:, h * 1024 : (h + 1) * 1024]
                )
                dst_ap = vaug[:, h * 1024 : (h + 1) * 1024].rearrange(
                    "p (t c) -> p t c", c=128
                )[:, :, 0:H]
                nc.vector.tensor_copy(dst_ap, vtr_h)

            def unit(p, j):
                # scores + exp for t-blocks (2p, 2p+1) x q-cols [512j, 512j+512)
                u = ps.tile([128, 1024], F32, tag="ps")
                tbE, tbO = 2 * p, 2 * p + 1
                nc.tensor.matmul(
                    u[:, 0:512],
                    lhsT=kiT[0:H, tbE * 128 : (tbE + 1) * 128],
                    rhs=qiT[0:H, j * 512 : (j + 1) * 512],
                    start=True,
                    stop=True,
                )
                nc.tensor.matmul(
                    u[:, 512:1024],
                    lhsT=kiT[H:128, tbO * 128 : (tbO + 1) * 128],
                    rhs=qiT[H:128, j * 512 : (j + 1) * 512],
                    start=True,
                    stop=True,
                )
                nc.scalar.activation(
                    att[:, p * 4096 + j * 1024 : p * 4096 + (j + 1) * 1024],
                    u[:],
                    mybir.ActivationFunctionType.Exp,
                    scale=INV_SQRT_C,
                )

            def outs(tbs, js):
                for j in js:
                    for tb in tbs:
                        nc.tensor.matmul(
                            po[:, j * 512 : (j + 1) * 512],
                            lhsT=vaug[:, tb * 128 : (tb + 1) * 128],
                            rhs=att[
                                :,
                                (tb // 2) * 4096
                                + j * 1024
                                + (tb % 2) * 512 : (tb // 2) * 4096
                                + j * 1024
                                + (tb % 2) * 512
                                + 512,
                            ],
                            start=(tb == 0),
                            stop=(tb == TB - 1),
                        )
                        out_emitted[j] += 1
                    if out_emitted[j] == TB:
                        norm(j)

            def norm(j):
                jc = slice(j * 512, (j + 1) * 512)
                nc.vector.tensor_copy(dsb[:, jc], po[H:128, jc])
                nc.vector.reciprocal_approx_fast(rec[:, jc], dsb[:, jc])
                nc.vector.tensor_tensor(
                    out_sb[:, jc], po[0:H, jc], rec[:, jc], op=mybir.AluOpType.mult
                )
                nc.sync.dma_start(outT_d[:, jc], out_sb[:, jc])

            # ---- PE stream, woven so the in-order queue never blocks ----
            proj_chunk(q_in, wq_t, bq_t, qiT, 0)
            proj_chunk(k_in, wk_t, bk_t, kiT, 0)
            unit(0, 0)
            unit(1, 0)
            proj_chunk(q_in, wq_t, bq_t, qiT, 1)
            unit(0, 1)
            unit(1, 1)

            proj_chunk(k_in, wk_t, bk_t, kiT, 1)
            unit(2, 0)
            unit(3, 0)
            unit(2, 1)
            unit(3, 1)

            proj_chunk(k_in, wk_t, bk_t, kiT, 2)
            unit(4, 0)
            unit(5, 0)
            unit(4, 1)
            unit(5, 1)
            proj_v(0)
            outs(range(0, 4), (0, 1))

            proj_chunk(k_in, wk_t, bk_t, kiT, 3)
            unit(6, 0)
            unit(7, 0)
            unit(6, 1)
            unit(7, 1)
            proj_v(1)
            outs(range(4, 8), (0, 1))

            proj_chunk(q_in, wq_t, bq_t, qiT, 2)
            unit(0, 2)
            unit(1, 2)
            outs(range(8, 12), (0, 1))
            unit(2, 2)
            unit(3, 2)
            outs(range(12, 16), (0, 1))
            unit(4, 2)
            unit(5, 2)
            outs(range(0, 4), (2,))
            unit(6, 2)
            unit(7, 2)
            outs(range(4, 8), (2,))

            proj_chunk(q_in, wq_t, bq_t, qiT, 3)
            unit(0, 3)
            unit(1, 3)
            outs(range(8, 12), (2,))
            unit(2, 3)
            unit(3, 3)
            outs(range(12, 16), (2,))
            unit(4, 3)
            unit(5, 3)
            outs(range(0, 8), (3,))
            unit(6, 3)
            unit(7, 3)
            outs(range(8, 16), (3,))

    nc.compile()
    return nc


def _prep_inputs(q, k, v, Wq, bq, Wk, bk, Wv, bv):
    """Host-side layout prep: per-batch transpose + fp16 cast."""
    w2 = {}
    for name, W in (("wq", Wq), ("wk", Wk)):
        w2[name] = np.ascontiguousarray(
            np.concatenate([W, W], axis=1), dtype=np.float16
        )
    w2["wv"] = np.ascontiguousarray(Wv, dtype=np.float16)
    b2 = {
        "bq": np.ascontiguousarray(
            np.tile(np.asarray(bq, dtype=np.float32).reshape(H, 1), (2, 1))
        ),
        "bk": np.ascontiguousarray(
            np.tile(np.asarray(bk, dtype=np.float32).reshape(H, 1), (2, 1))
        ),
        "bv": np.ascontiguousarray(np.asarray(bv, dtype=np.float32).reshape(H, 1)),
    }
    in_maps = []
    for i in range(B):
        m = {
            "qT": np.ascontiguousarray(q[i].T, dtype=np.float16),
            "kT": np.ascontiguousarray(k[i].T, dtype=np.float16),
            "vT": np.ascontiguousarray(v[i].T, dtype=np.float16),
        }
        m.update(w2)
        m.update(b2)
        in_maps.append(m)
    return in_maps


def run(trace=False, **inputs):
    """Build (cached), run on 8 cores, gather. Returns (out, BassKernelResults)."""
    if "nc" not in _CACHE:
        _CACHE["nc"] = build_program()
    nc = _CACHE["nc"]
    in_maps = _prep_inputs(**{k2: np.asarray(v2) for k2, v2 in inputs.items()})
    res = run_bass_kernel_spmd(nc, in_maps, list(range(B)), trace=trace)
    out = np.stack([np.ascontiguousarray(res.results[i]["outT"].T) for i in range(B)])
    return out.astype(np.float32), res


def kernel(**inputs) -> np.ndarray:
    out, _ = run(trace=False, **inputs)
    return out


# revision 3
# speedup vs baseline: 1.1258x; 1.0519x over previous
"""Single-head attention kernel for Trainium2, SPMD over 8 NeuronCores.

Problem: out = softmax((q@Wq+bq) @ (k@Wk+bk)^T / sqrt(768)) @ (v@Wv+bv)
Shapes: q,k,v [8, 2048, 768] fp32; W* [768, 64]; b* [64].

Strategy (v3): data-parallel over batch (1 batch per core). The kernel is a
software-pipelined stream bounded by the ScalarE exp wall (4.2M exps ~30us):

  - Host prep re-chunks all inputs so every DMA moves >=6KB contiguous per
    partition row (small-element DMAs run at ~70GB/s vs 358GB/s for 2KB+):
    q/k arrive as 4 chunks of [128, 6x512] (one 512-column s-chunk, all six
    e-chunks packed per row), v as 2 chunks of [128, 6x1024], all weights in
    one [128, 1920] tile, biases in one [128, 4].
  - Two HWDGE queues: Scalar carries the critical gate (weights, q chunk 0)
    plus the vi transposes; Sync carries everything else in need-time order.
  - A few dummy matmuls on the weight tile warm the PE HAM clock gate while
    the first input chunks stream in.
  - projections: x@W via W.T-as-lhsT accumulated over 6 e-chunks. wq/wk are
    duplicated [768,128] so qiT/kiT hold 2 identical copies across the 128
    partitions (feeds both row-tile groups).
  - scores in (pair, j) units: pair p = t-blocks (2p, 2p+1), j = 512
    q-columns. Two K=64 matmuls run CONCURRENTLY in PE row groups 0-63 /
    64-127 (tile_position row tiling), filling the two halves of one
    [128,1024] PSUM tile; a single FD-1024 exp on ScalarE (scale 1/sqrt(768)
    fused; scaled scores are N(0,1/12) so no max-subtraction needed).
  - v: viT = Wv.T @ vT, then one batched DMA-xbar transpose per half into a
    contiguous scratch, DVE-copied into vaug = [vi | ones] blocks (the ones
    make PSUM rows 64-127 of the output accumulate the softmax denominator).
  - output: per (t-block, j) matmul into a persistent [128,2048] PSUM
    region, woven into the score stream with a lag so the in-order PE queue
    never blocks on late operands.
  - normalize per 512-column chunk as soon as its accumulation stops (copy
    denom, reciprocal_approx_fast, multiply, DMA out) so the tail after the
    last exp is ~2.5us.
"""

import numpy as np
from contextlib import ExitStack

import concourse.bass as bass
import concourse.mybir as mybir
import concourse.tile as tile
from concourse import bacc
from concourse.bass_utils import run_bass_kernel_spmd

E = 768  # n_embd
H = 64  # head size
S = 2048  # sequence length
B = 8  # batch == n_cores
EC = E // 128  # e chunks
TB = S // 128  # t blocks
NP = TB // 2  # t-block pairs
INV_SQRT_C = float(1.0 / np.sqrt(np.float32(E)))
WQK = EC * 128  # 768 packed weight cols per q/k tensor
WV = EC * H  # 384 packed weight cols for v

F16 = mybir.dt.float16
F32 = mybir.dt.float32

_CACHE = {}


def build_program():
    nc = bacc.Bacc(
        "TRN2",
        target_bir_lowering=False,
        debug=False,
        enable_asserts=False,
        num_devices=B,
    )

    # chunk-major host layouts: per-partition rows are >=6KB contiguous
    qT_d = nc.dram_tensor("qTc", [4 * 128, EC * 512], F16, kind="ExternalInput")
    kT_d = nc.dram_tensor("kTc", [4 * 128, EC * 512], F16, kind="ExternalInput")
    vT_d = nc.dram_tensor("vTc", [2 * 128, EC * 1024], F16, kind="ExternalInput")
    w_d = nc.dram_tensor("wpack", [128, 2 * WQK + WV], F16, kind="ExternalInput")
    b_d = nc.dram_tensor("bpack", [128, 4], F32, kind="ExternalInput")
    outT_d = nc.dram_tensor("outT", [H, S], F32, kind="ExternalOutput")

    with tile.TileContext(nc) as tc, ExitStack() as ctx:
        const = ctx.enter_context(tc.tile_pool(name="const", bufs=1))
        xin = ctx.enter_context(tc.tile_pool(name="xin", bufs=1))
        acts = ctx.enter_context(tc.tile_pool(name="acts", bufs=1))

        w_t = const.tile([128, 2 * WQK + WV], F16, tag="w")
        b_t = const.tile([128, 4], F32, tag="b")
        warm = const.tile([128, 8], F32, tag="warm")
        wq_t = w_t[:, 0:WQK]
        wk_t = w_t[:, WQK : 2 * WQK]
        wv_t = w_t[:, 2 * WQK : 2 * WQK + WV]
        bq_t = b_t[:, 0:1]
        bk_t = b_t[:, 1:2]
        bv_t = b_t[0:H, 2:3]

        q_in = xin.tile([128, 4 * EC * 512], F16, tag="q_in")
        k_in = xin.tile([128, 4 * EC * 512], F16, tag="k_in")
        v_in = xin.tile([128, 2 * EC * 1024], F16, tag="v_in")

        qiT = acts.tile([128, S], F16, tag="qiT")
        kiT = acts.tile([128, S], F16, tag="kiT")
        viT = acts.tile([H, S], F16, tag="viT")
        vtr = acts.tile([128, S // 2], F16, tag="vtr")  # [128, 8, 64] x2 halves
        vaug = acts.tile([128, TB * 128], F16, tag="vaug")
        att = acts.tile([128, NP * 4096], F16, tag="att")
        dsb = acts.tile([H, S], F32, tag="dsb")
        rec = acts.tile([H, S], F32, tag="rec")
        out_sb = acts.tile([H, S], F32, tag="out_sb")

        def qk_chunk(x_t, j):
            # [128, EC, 512] view of s-columns [512j, 512j+512)
            return x_t[:, j * EC * 512 : (j + 1) * EC * 512].rearrange(
                "p (e w) -> p e w", w=512
            )

        def v_chunk(h):
            return v_in[:, h * EC * 1024 : (h + 1) * EC * 1024].rearrange(
                "p (e w) -> p e w", w=1024
            )

        def load_qk(eng, x_t, x_d, j):
            eng.dma_start(
                x_t[:, j * EC * 512 : (j + 1) * EC * 512],
                x_d[j * 128 : (j + 1) * 128, :],
            )

        def load_v(eng, h):
            eng.dma_start(
                v_in[:, h * EC * 1024 : (h + 1) * EC * 1024],
                vT_d[h * 128 : (h + 1) * 128, :],
            )

        # ---- Scalar HWDGE queue: the critical gate, then exp-table warm ----
        nc.vector.memset(warm[:], 0.0)
        nc.scalar.dma_start(w_t[:], w_d[:])
        load_qk(nc.scalar, q_in, qT_d, 0)
        nc.scalar.dma_start(b_t[:], b_d[:])
        nc.scalar.activation(
            warm[:], warm[:], mybir.ActivationFunctionType.Exp, scale=1.0
        )

        # ---- Sync HWDGE queue: everything else, in need-time order ----
        load_qk(nc.sync, k_in, kT_d, 0)
        load_qk(nc.sync, q_in, qT_d, 1)
        load_qk(nc.sync, k_in, kT_d, 1)
        load_v(nc.sync, 0)
        load_qk(nc.sync, k_in, kT_d, 2)

        nc.vector.memset(vaug[:], 1.0)

        with tc.tile_pool(name="ps", bufs=2, space="PSUM") as ps, tc.tile_pool(
            name="op", bufs=1, space="PSUM"
        ) as op:
            po = op.tile([128, S], F32, tag="po")
            out_emitted = [0, 0, 0, 0]

            # ---- PE HAM warm-up on the weight tile while inputs stream ----
            for i in range(8):
                pw = ps.tile([128, 1024], F32, tag="ps")
                nc.tensor.matmul(
                    pw[:, 0:512],
                    lhsT=w_t[:, (i % 6) * 128 : (i % 6) * 128 + 128],
                    rhs=w_t[:, 0:512],
                    start=True,
                    stop=True,
                )

            def proj_chunk(x_t, w_sl, b_sl, dst, c):
                # 512 s-columns of a q/k projection (dup'd weights)
                pj = ps.tile([128, 1024], F32, tag="ps")
                xc = qk_chunk(x_t, c)
                for e in range(EC):
                    nc.tensor.matmul(
                        pj[:, 0:512],
                        lhsT=w_sl[:, e * 128 : (e + 1) * 128],
                        rhs=xc[:, e, :],
                        start=(e == 0),
                        stop=(e == EC - 1),
                    )
                nc.vector.tensor_scalar_add(
                    dst[:, c * 512 : (c + 1) * 512], pj[:, 0:512], b_sl
                )

            def proj_v(h):
                # 1024 s-columns of the v projection (single-width weights)
                pj = ps.tile([128, 1024], F32, tag="ps")
                xc = v_chunk(h)
                for e in range(EC):
                    for n in range(2):
                        nc.tensor.matmul(
                            pj[0:H, n * 512 : (n + 1) * 512],
                            lhsT=wv_t[:, e * H : (e + 1) * H],
                            rhs=xc[:, e, n * 512 : (n + 1) * 512],
                            start=(e == 0),
                            stop=(e == EC - 1),
                        )
                nc.vector.tensor_scalar_add(
                    viT[:, h * 1024 : (h + 1) * 1024], pj[0:H, :], bv_t
                )
                # viT[64,1024] -> vtr [128, 8, 64] via DMA xbar transpose on
                # the Scalar queue (issue-only engine cost; contiguous dst —
                # a strided dst is a HW bug), then DVE-copy into the
                # [vi | ones] vaug blocks.
                vtr_h = vtr[:, h * 512 : (h + 1) * 512].rearrange(
                    "p (t k) -> p t k", k=H
                )
                nc.scalar.dma_start_transpose(
                    vtr_h, viT[:, h * 1024 : (h + 1) * 1024]
                )
                dst_ap = vaug[:, h * 1024 : (h + 1) * 1024].rearrange(
                    "p (t c) -> p t c", c=128
                )[:, :, 0:H]
                nc.vector.tensor_copy(dst_ap, vtr_h)

            def unit(p, j):
                # scores + exp for t-blocks (2p, 2p+1) x q-cols [512j, 512j+512)
                u = ps.tile([128, 1024], F32, tag="ps")
                tbE, tbO = 2 * p, 2 * p + 1
                nc.tensor.matmul(
                    u[:, 0:512],
                    lhsT=kiT[0:H, tbE * 128 : (tbE + 1) * 128],
                    rhs=qiT[0:H, j * 512 : (j + 1) * 512],
                    start=True,
                    stop=True,
                )
                nc.tensor.matmul(
                    u[:, 512:1024],
                    lhsT=kiT[H:128, tbO * 128 : (tbO + 1) * 128],
                    rhs=qiT[H:128, j * 512 : (j + 1) * 512],
                    start=True,
                    stop=True,
                )
                nc.scalar.activation(
                    att[:, p * 4096 + j * 1024 : p * 4096 + (j + 1) * 1024],
                    u[:],
                    mybir.ActivationFunctionType.Exp,
                    scale=INV_SQRT_C,
                )

            def outs(tbs, js):
                for j in js:
                    for tb in tbs:
                        base = (tb // 2) * 4096 + j * 1024 + (tb % 2) * 512
                        nc.tensor.matmul(
                            po[:, j * 512 : (j + 1) * 512],
                            lhsT=vaug[:, tb * 128 : (tb + 1) * 128],
                            rhs=att[:, base : base + 512],
                            start=(tb == 0),
                            stop=(tb == TB - 1),
                        )
                        out_emitted[j] += 1
                    if out_emitted[j] == TB:
                        norm(j)

            def norm(j):
                jc = slice(j * 512, (j + 1) * 512)
                nc.vector.tensor_copy(dsb[:, jc], po[H:128, jc])
                nc.vector.reciprocal_approx_fast(rec[:, jc], dsb[:, jc])
                nc.vector.tensor_tensor(
                    out_sb[:, jc], po[0:H, jc], rec[:, jc], op=mybir.AluOpType.mult
                )
                nc.sync.dma_start(outT_d[:, jc], out_sb[:, jc])

            # ---- PE stream, woven so the in-order queue never blocks ----
            proj_chunk(q_in, wq_t, bq_t, qiT, 0)
            proj_chunk(k_in, wk_t, bk_t, kiT, 0)
            unit(0, 0)
            unit(1, 0)
            proj_chunk(q_in, wq_t, bq_t, qiT, 1)
            unit(0, 1)
            unit(1, 1)

            proj_chunk(k_in, wk_t, bk_t, kiT, 1)
            unit(2, 0)
            unit(3, 0)
            unit(2, 1)
            unit(3, 1)
            proj_v(0)

            load_qk(nc.sync, k_in, kT_d, 3)
            load_v(nc.sync, 1)

            proj_chunk(k_in, wk_t, bk_t, kiT, 2)
            unit(4, 0)
            unit(5, 0)
            unit(4, 1)
            unit(5, 1)
            outs(range(0, 4), (0, 1))

            proj_chunk(k_in, wk_t, bk_t, kiT, 3)
            proj_v(1)
            unit(6, 0)
            unit(7, 0)
            unit(6, 1)
            unit(7, 1)
            outs(range(4, 8), (0, 1))

            load_qk(nc.sync, q_in, qT_d, 2)
            load_qk(nc.sync, q_in, qT_d, 3)

            proj_chunk(q_in, wq_t, bq_t, qiT, 2)
            unit(0, 2)
            unit(1, 2)
            outs(range(8, 12), (0, 1))
            unit(2, 2)
            unit(3, 2)
            outs(range(12, 16), (0, 1))
            unit(4, 2)
            unit(5, 2)
            outs(range(0, 4), (2,))
            unit(6, 2)
            unit(7, 2)
            outs(range(4, 8), (2,))

            proj_chunk(q_in, wq_t, bq_t, qiT, 3)
            unit(0, 3)
            unit(1, 3)
            outs(range(8, 12), (2,))
            unit(2, 3)
            unit(3, 3)
            outs(range(12, 16), (2,))
            unit(4, 3)
            unit(5, 3)
            outs(range(0, 8), (3,))
            unit(6, 3)
            unit(7, 3)
            outs(range(8, 16), (3,))

    nc.compile()
    return nc


def _prep_inputs(q, k, v, Wq, bq, Wk, bk, Wv, bv):
    """Host-side layout prep: per-batch transpose + fp16 cast + re-chunk."""
    wq2 = np.concatenate([Wq, Wq], axis=1).astype(np.float16)  # [768, 128]
    wk2 = np.concatenate([Wk, Wk], axis=1).astype(np.float16)
    wv1 = np.asarray(Wv, dtype=np.float16)  # [768, 64]
    wpack = np.ascontiguousarray(
        np.concatenate(
            [
                wq2.reshape(EC, 128, 128).transpose(1, 0, 2).reshape(128, WQK),
                wk2.reshape(EC, 128, 128).transpose(1, 0, 2).reshape(128, WQK),
                wv1.reshape(EC, 128, H).transpose(1, 0, 2).reshape(128, WV),
            ],
            axis=1,
        )
    )
    bpack = np.zeros((128, 4), dtype=np.float32)
    bpack[:, 0] = np.tile(np.asarray(bq, np.float32), 2)
    bpack[:, 1] = np.tile(np.asarray(bk, np.float32), 2)
    bpack[:, 2] = np.tile(np.asarray(bv, np.float32), 2)

    def chunk(xT, w):
        # [768, 2048] -> [(S/w)*128, 6*w]: s-chunk-major, e-chunks packed
        return np.ascontiguousarray(
            xT.reshape(EC, 128, S // w, w)
            .transpose(2, 1, 0, 3)
            .reshape((S // w) * 128, EC * w)
        )

    in_maps = []
    for i in range(B):
        qT = np.asarray(q[i], np.float16).T
        kT = np.asarray(k[i], np.float16).T
        vT = np.asarray(v[i], np.float16).T
        m = {
            "qTc": chunk(qT, 512),
            "kTc": chunk(kT, 512),
            "vTc": chunk(vT, 1024),
            "wpack": wpack,
            "bpack": bpack,
        }
        in_maps.append(m)
    return in_maps


def run(trace=False, **inputs):
    """Build (cached), run on 8 cores, gather. Returns (out, BassKernelResults)."""
    if "nc" not in _CACHE:
        _CACHE["nc"] = build_program()
    nc = _CACHE["nc"]
    in_maps = _prep_inputs(**{k2: np.asarray(v2) for k2, v2 in inputs.items()})
    res = run_bass_kernel_spmd(nc, in_maps, list(range(B)), trace=trace)
    out = np.stack([np.ascontiguousarray(res.results[i]["outT"].T) for i in range(B)])
    return out.astype(np.float32), res


def kernel(**inputs) -> np.ndarray:
    out, _ = run(trace=False, **inputs)
    return out


# revision 4
# speedup vs baseline: 1.2197x; 1.0834x over previous
"""Single-head attention kernel for Trainium2, SPMD over 8 NeuronCores.

Problem: out = softmax((q@Wq+bq) @ (k@Wk+bk)^T / sqrt(768)) @ (v@Wv+bv)
Shapes: q,k,v [8, 2048, 768] fp32; W* [768, 64]; b* [64].

Strategy (v4): data-parallel over batch (1 batch per core). The kernel is a
software-pipelined stream bounded by the ScalarE exp wall (4.2M exps ~30us):

  - Host prep re-chunks all inputs so every DMA moves >=3KB contiguous per
    partition row (small-element DMAs run far below the ~360GB/s peak):
    q/k arrive as 8 chunks of [128, 6x256], v as 2 chunks of [128, 6x1024],
    all weights + a 64x64 identity in one [128, 1984] tile, biases in one
    [128, 4].
  - Two HWDGE queues, ordered by need-time with a minimal critical gate
    (weights + first 256-col q/k chunks ~0.9MB) so the exp wall starts
    around 12us instead of 27us.
  - A few dummy matmuls on the weight tile warm the PE HAM clock gate while
    the first input chunks stream in.
  - projections: x@W via W.T-as-lhsT accumulated over 6 e-chunks. wq/wk are
    duplicated [768,128] so qiT/kiT hold 2 identical copies across the 128
    partitions (feeds both row-tile groups).
  - scores in (pair, j) units: pair p = t-blocks (2p, 2p+1), j = 512
    q-columns. Two K=64 matmuls run CONCURRENTLY in PE row groups 0-63 /
    64-127 (tile_position row tiling), filling the two halves of one
    [128,1024] PSUM tile; a single FD-1024 exp on ScalarE (scale 1/sqrt(768)
    fused; scaled scores are N(0,1/12) so no max-subtraction needed).
  - v: viT = Wv.T @ vT, then PE transpose-mode matmuls against the identity
    (in the PE stream, so no cross-engine queue blocking), DVE-copied into
    vaug = [vi | ones] blocks (the ones make PSUM rows 64-127 of the output
    accumulate the softmax denominator for free).
  - output: per (t-block, j) matmul into a persistent [128,2048] PSUM
    region, woven into the score stream with a lag so the in-order PE queue
    never blocks on late operands.
  - normalize per 512-column chunk as soon as its accumulation stops (copy
    denom, reciprocal_approx_fast, multiply, DMA out) so the tail after the
    last exp is ~2.5us.
"""

import numpy as np
from contextlib import ExitStack

import concourse.bass as bass
import concourse.mybir as mybir
import concourse.tile as tile
from concourse import bacc
from concourse.bass_utils import run_bass_kernel_spmd

E = 768  # n_embd
H = 64  # head size
S = 2048  # sequence length
B = 8  # batch == n_cores
EC = E // 128  # e chunks
TB = S // 128  # t blocks
NP = TB // 2  # t-block pairs
INV_SQRT_C = float(1.0 / np.sqrt(np.float32(E)))
WQK = EC * 128  # 768 packed weight cols per q/k tensor
WV = EC * H  # 384 packed weight cols for v
CW = 256  # q/k DMA+projection chunk width
NC_ = S // CW  # 8 chunks

F16 = mybir.dt.float16
F32 = mybir.dt.float32

_CACHE = {}


def build_program():
    nc = bacc.Bacc(
        "TRN2",
        target_bir_lowering=False,
        debug=False,
        enable_asserts=False,
        num_devices=B,
    )

    # chunk-major host layouts: per-partition rows are >=3KB contiguous
    qT_d = nc.dram_tensor("qTc", [NC_ * 128, EC * CW], F16, kind="ExternalInput")
    kT_d = nc.dram_tensor("kTc", [NC_ * 128, EC * CW], F16, kind="ExternalInput")
    vT_d = nc.dram_tensor("vTc", [2 * 128, EC * 1024], F16, kind="ExternalInput")
    w_d = nc.dram_tensor("wpack", [128, 2 * WQK + WV + H], F16, kind="ExternalInput")
    b_d = nc.dram_tensor("bpack", [128, 4], F32, kind="ExternalInput")
    outT_d = nc.dram_tensor("outT", [H, S], F32, kind="ExternalOutput")

    with tile.TileContext(nc) as tc, ExitStack() as ctx:
        const = ctx.enter_context(tc.tile_pool(name="const", bufs=1))
        xin = ctx.enter_context(tc.tile_pool(name="xin", bufs=1))
        acts = ctx.enter_context(tc.tile_pool(name="acts", bufs=1))

        w_t = const.tile([128, 2 * WQK + WV + H], F16, tag="w")
        b_t = const.tile([128, 4], F32, tag="b")
        warm = const.tile([128, 8], F32, tag="warm")
        wq_t = w_t[:, 0:WQK]
        wk_t = w_t[:, WQK : 2 * WQK]
        wv_t = w_t[:, 2 * WQK : 2 * WQK + WV]
        id_t = w_t[0:H, 2 * WQK + WV : 2 * WQK + WV + H]
        bq_t = b_t[:, 0:1]
        bk_t = b_t[:, 1:2]
        bv_t = b_t[0:H, 2:3]

        q_in = xin.tile([128, S * EC], F16, tag="q_in")
        k_in = xin.tile([128, S * EC], F16, tag="k_in")
        v_in = xin.tile([128, 2 * EC * 1024], F16, tag="v_in")

        qiT = acts.tile([128, S], F16, tag="qiT")
        kiT = acts.tile([128, S], F16, tag="kiT")
        viT = acts.tile([H, S], F16, tag="viT")
        vaug = acts.tile([128, TB * 128], F16, tag="vaug")
        att = acts.tile([128, NP * 4096], F16, tag="att")
        dsb = acts.tile([H, S], F32, tag="dsb")
        rec = acts.tile([H, S], F32, tag="rec")
        out_sb = acts.tile([H, S], F32, tag="out_sb")

        def qk_chunk(x_t, c):
            # [128, EC, CW] view of s-columns [CW*c, CW*(c+1))
            return x_t[:, c * EC * CW : (c + 1) * EC * CW].rearrange(
                "p (e w) -> p e w", w=CW
            )

        def v_chunk(h):
            return v_in[:, h * EC * 1024 : (h + 1) * EC * 1024].rearrange(
                "p (e w) -> p e w", w=1024
            )

        def load_qk(eng, x_t, x_d, c):
            eng.dma_start(
                x_t[:, c * EC * CW : (c + 1) * EC * CW],
                x_d[c * 128 : (c + 1) * 128, :],
            )

        def load_v(eng, h):
            eng.dma_start(
                v_in[:, h * EC * 1024 : (h + 1) * EC * 1024],
                vT_d[h * 128 : (h + 1) * 128, :],
            )

        # ---- Scalar HWDGE queue: the critical gate, then exp-table warm ----
        nc.vector.memset(warm[:], 0.0)
        nc.scalar.dma_start(w_t[:], w_d[:])
        load_qk(nc.scalar, q_in, qT_d, 0)
        load_qk(nc.scalar, q_in, qT_d, 1)
        nc.scalar.dma_start(b_t[:], b_d[:])
        nc.scalar.activation(
            warm[:], warm[:], mybir.ActivationFunctionType.Exp, scale=1.0
        )

        # ---- Sync HWDGE queue: everything else, in need-time order ----
        load_qk(nc.sync, k_in, kT_d, 0)
        load_qk(nc.sync, k_in, kT_d, 1)
        load_qk(nc.sync, q_in, qT_d, 2)
        load_qk(nc.sync, q_in, qT_d, 3)
        load_qk(nc.sync, k_in, kT_d, 2)
        load_qk(nc.sync, k_in, kT_d, 3)
        load_v(nc.sync, 0)

        nc.vector.memset(vaug[:], 1.0)

        with tc.tile_pool(name="ps", bufs=2, space="PSUM") as ps, tc.tile_pool(
            name="op", bufs=1, space="PSUM"
        ) as op:
            po = op.tile([128, S], F32, tag="po")
            out_emitted = [0, 0, 0, 0]

            # ---- PE HAM warm-up on the weight tile while inputs stream ----
            for i in range(6):
                pw = ps.tile([128, 1024], F32, tag="ps")
                nc.tensor.matmul(
                    pw[:, 0:512],
                    lhsT=w_t[:, (i % 6) * 128 : (i % 6) * 128 + 128],
                    rhs=w_t[:, 0:512],
                    start=True,
                    stop=True,
                )

            def proj_chunk(x_t, w_sl, b_sl, dst, c):
                # CW s-columns of a q/k projection (dup'd weights)
                pj = ps.tile([128, 1024], F32, tag="ps")
                xc = qk_chunk(x_t, c)
                for e in range(EC):
                    nc.tensor.matmul(
                        pj[:, 0:CW],
                        lhsT=w_sl[:, e * 128 : (e + 1) * 128],
                        rhs=xc[:, e, :],
                        start=(e == 0),
                        stop=(e == EC - 1),
                    )
                nc.vector.tensor_scalar_add(
                    dst[:, c * CW : (c + 1) * CW], pj[:, 0:CW], b_sl
                )

            def proj_v(h):
                # 1024 s-columns of the v projection (single-width weights)
                pj = ps.tile([128, 1024], F32, tag="ps")
                xc = v_chunk(h)
                for e in range(EC):
                    for n in range(2):
                        nc.tensor.matmul(
                            pj[0:H, n * 512 : (n + 1) * 512],
                            lhsT=wv_t[:, e * H : (e + 1) * H],
                            rhs=xc[:, e, n * 512 : (n + 1) * 512],
                            start=(e == 0),
                            stop=(e == EC - 1),
                        )
                nc.vector.tensor_scalar_add(
                    viT[:, h * 1024 : (h + 1) * 1024], pj[0:H, :], bv_t
                )

            def tr_v(h):
                # viT [64, 8x128] -> vi [128, 8, 64] via PE transpose-mode,
                # then DVE-copy into the [vi | ones] vaug blocks
                tr = ps.tile([128, 512], F16, tag="ps")
                for i in range(8):
                    tb = h * 8 + i
                    nc.tensor.transpose(
                        tr[:, i * H : (i + 1) * H],
                        viT[:, tb * 128 : (tb + 1) * 128],
                        id_t,
                    )
                dst_ap = vaug[:, h * 1024 : (h + 1) * 1024].rearrange(
                    "p (t c) -> p t c", c=128
                )[:, :, 0:H]
                nc.vector.tensor_copy(
                    dst_ap, tr[:].rearrange("p (t k) -> p t k", k=H)
                )

            def unit(p, j):
                # scores + exp for t-blocks (2p, 2p+1) x q-cols [512j, 512j+512)
                u = ps.tile([128, 1024], F32, tag="ps")
                tbE, tbO = 2 * p, 2 * p + 1
                nc.tensor.matmul(
                    u[:, 0:512],
                    lhsT=kiT[0:H, tbE * 128 : (tbE + 1) * 128],
                    rhs=qiT[0:H, j * 512 : (j + 1) * 512],
                    start=True,
                    stop=True,
                )
                nc.tensor.matmul(
                    u[:, 512:1024],
                    lhsT=kiT[H:128, tbO * 128 : (tbO + 1) * 128],
                    rhs=qiT[H:128, j * 512 : (j + 1) * 512],
                    start=True,
                    stop=True,
                )
                nc.scalar.activation(
                    att[:, p * 4096 + j * 1024 : p * 4096 + (j + 1) * 1024],
                    u[:],
                    mybir.ActivationFunctionType.Exp,
                    scale=INV_SQRT_C,
                )

            def outs(tbs, js):
                for j in js:
                    for tb in tbs:
                        base = (tb // 2) * 4096 + j * 1024 + (tb % 2) * 512
                        nc.tensor.matmul(
                            po[:, j * 512 : (j + 1) * 512],
                            lhsT=vaug[:, tb * 128 : (tb + 1) * 128],
                            rhs=att[:, base : base + 512],
                            start=(tb == 0),
                            stop=(tb == TB - 1),
                        )
                        out_emitted[j] += 1
                    if out_emitted[j] == TB:
                        norm(j)

            def norm(j):
                jc = slice(j * 512, (j + 1) * 512)
                nc.vector.tensor_copy(dsb[:, jc], po[H:128, jc])
                nc.vector.reciprocal_approx_fast(rec[:, jc], dsb[:, jc])
                nc.vector.tensor_tensor(
                    out_sb[:, jc], po[0:H, jc], rec[:, jc], op=mybir.AluOpType.mult
                )
                nc.sync.dma_start(outT_d[:, jc], out_sb[:, jc])

            # ---- PE stream, woven so the in-order queue never blocks ----
            proj_chunk(q_in, wq_t, bq_t, qiT, 0)
            proj_chunk(q_in, wq_t, bq_t, qiT, 1)
            proj_chunk(k_in, wk_t, bk_t, kiT, 0)
            proj_chunk(k_in, wk_t, bk_t, kiT, 1)
            unit(0, 0)
            unit(1, 0)
            proj_chunk(q_in, wq_t, bq_t, qiT, 2)
            proj_chunk(q_in, wq_t, bq_t, qiT, 3)
            unit(0, 1)
            unit(1, 1)

            proj_chunk(k_in, wk_t, bk_t, kiT, 2)
            proj_chunk(k_in, wk_t, bk_t, kiT, 3)
            unit(2, 0)
            unit(3, 0)
            unit(2, 1)
            unit(3, 1)
            proj_v(0)

            load_qk(nc.sync, k_in, kT_d, 4)
            load_qk(nc.sync, k_in, kT_d, 5)
            load_v(nc.sync, 1)

            proj_chunk(k_in, wk_t, bk_t, kiT, 4)
            proj_chunk(k_in, wk_t, bk_t, kiT, 5)
            unit(4, 0)
            unit(5, 0)
            tr_v(0)
            unit(4, 1)
            unit(5, 1)
            outs(range(0, 4), (0, 1))

            load_qk(nc.sync, k_in, kT_d, 6)
            load_qk(nc.sync, k_in, kT_d, 7)

            proj_chunk(k_in, wk_t, bk_t, kiT, 6)
            proj_chunk(k_in, wk_t, bk_t, kiT, 7)
            proj_v(1)
            unit(6, 0)
            unit(7, 0)
            tr_v(1)
            unit(6, 1)
            unit(7, 1)
            outs(range(4, 8), (0, 1))

            load_qk(nc.sync, q_in, qT_d, 4)
            load_qk(nc.sync, q_in, qT_d, 5)
            load_qk(nc.sync, q_in, qT_d, 6)
            load_qk(nc.sync, q_in, qT_d, 7)

            proj_chunk(q_in, wq_t, bq_t, qiT, 4)
            proj_chunk(q_in, wq_t, bq_t, qiT, 5)
            unit(0, 2)
            unit(1, 2)
            outs(range(8, 12), (0, 1))
            proj_chunk(q_in, wq_t, bq_t, qiT, 6)
            proj_chunk(q_in, wq_t, bq_t, qiT, 7)
            unit(2, 2)
            unit(3, 2)
            outs(range(12, 16), (0, 1))
            unit(4, 2)
            unit(5, 2)
            outs(range(0, 4), (2,))
            unit(6, 2)
            unit(7, 2)
            outs(range(4, 8), (2,))

            unit(0, 3)
            unit(1, 3)
            outs(range(8, 12), (2,))
            unit(2, 3)
            unit(3, 3)
            outs(range(12, 16), (2,))
            unit(4, 3)
            unit(5, 3)
            outs(range(0, 8), (3,))
            unit(6, 3)
            unit(7, 3)
            outs(range(8, 16), (3,))

    nc.compile()
    return nc


def _prep_inputs(q, k, v, Wq, bq, Wk, bk, Wv, bv):
    """Host-side layout prep: per-batch transpose + fp16 cast + re-chunk."""
    wq2 = np.concatenate([Wq, Wq], axis=1).astype(np.float16)  # [768, 128]
    wk2 = np.concatenate([Wk, Wk], axis=1).astype(np.float16)
    wv1 = np.asarray(Wv, dtype=np.float16)  # [768, 64]
    idp = np.zeros((128, H), dtype=np.float16)
    idp[0:H] = np.eye(H, dtype=np.float16)
    wpack = np.ascontiguousarray(
        np.concatenate(
            [
                wq2.reshape(EC, 128, 128).transpose(1, 0, 2).reshape(128, WQK),
                wk2.reshape(EC, 128, 128).transpose(1, 0, 2).reshape(128, WQK),
                wv1.reshape(EC, 128, H).transpose(1, 0, 2).reshape(128, WV),
                idp,
            ],
            axis=1,
        )
    )
    bpack = np.zeros((128, 4), dtype=np.float32)
    bpack[:, 0] = np.tile(np.asarray(bq, np.float32), 2)
    bpack[:, 1] = np.tile(np.asarray(bk, np.float32), 2)
    bpack[:, 2] = np.tile(np.asarray(bv, np.float32), 2)

    def chunk(xT, w):
        # [768, 2048] -> [(S/w)*128, 6*w]: s-chunk-major, e-chunks packed
        return np.ascontiguousarray(
            xT.reshape(EC, 128, S // w, w)
            .transpose(2, 1, 0, 3)
            .reshape((S // w) * 128, EC * w)
        )

    in_maps = []
    for i in range(B):
        qT = np.asarray(q[i], np.float16).T
        kT = np.asarray(k[i], np.float16).T
        vT = np.asarray(v[i], np.float16).T
        m = {
            "qTc": chunk(qT, CW),
            "kTc": chunk(kT, CW),
            "vTc": chunk(vT, 1024),
            "wpack": wpack,
            "bpack": bpack,
        }
        in_maps.append(m)
    return in_maps


def run(trace=False, **inputs):
    """Build (cached), run on 8 cores, gather. Returns (out, BassKernelResults)."""
    if "nc" not in _CACHE:
        _CACHE["nc"] = build_program()
    nc = _CACHE["nc"]
    in_maps = _prep_inputs(**{k2: np.asarray(v2) for k2, v2 in inputs.items()})
    res = run_bass_kernel_spmd(nc, in_maps, list(range(B)), trace=trace)
    out = np.stack([np.ascontiguousarray(res.results[i]["outT"].T) for i in range(B)])
    return out.astype(np.float32), res


def kernel(**inputs) -> np.ndarray:
    out, _ = run(trace=False, **inputs)
    return out


# revision 5
# speedup vs baseline: 1.2367x; 1.0139x over previous
"""Single-head attention kernel for Trainium2, SPMD over 8 NeuronCores.

Problem: out = softmax((q@Wq+bq) @ (k@Wk+bk)^T / sqrt(768)) @ (v@Wv+bv)
Shapes: q,k,v [8, 2048, 768] fp32; W* [768, 64]; b* [64].

Strategy (v4): data-parallel over batch (1 batch per core). The kernel is a
software-pipelined stream bounded by the ScalarE exp wall (4.2M exps ~30us):

  - Host prep re-chunks all inputs so every DMA moves >=3KB contiguous per
    partition row (small-element DMAs run far below the ~360GB/s peak):
    q/k arrive as 8 chunks of [128, 6x256], v as 2 chunks of [128, 6x1024],
    all weights + a 64x64 identity in one [128, 1984] tile, biases in one
    [128, 4].
  - Two HWDGE queues, ordered by need-time with a minimal critical gate
    (weights + first 256-col q/k chunks ~0.9MB) so the exp wall starts
    around 12us instead of 27us.
  - A few dummy matmuls on the weight tile warm the PE HAM clock gate while
    the first input chunks stream in.
  - projections: x@W via W.T-as-lhsT accumulated over 6 e-chunks. wq/wk are
    duplicated [768,128] so qiT/kiT hold 2 identical copies across the 128
    partitions (feeds both row-tile groups).
  - scores in (pair, j) units: pair p = t-blocks (2p, 2p+1), j = 512
    q-columns. Two K=64 matmuls run CONCURRENTLY in PE row groups 0-63 /
    64-127 (tile_position row tiling), filling the two halves of one
    [128,1024] PSUM tile; a single FD-1024 exp on ScalarE (scale 1/sqrt(768)
    fused; scaled scores are N(0,1/12) so no max-subtraction needed).
  - v: viT = Wv.T @ vT, then PE transpose-mode matmuls against the identity
    (in the PE stream, so no cross-engine queue blocking), DVE-copied into
    vaug = [vi | ones] blocks (the ones make PSUM rows 64-127 of the output
    accumulate the softmax denominator for free).
  - output: per (t-block, j) matmul into a persistent [128,2048] PSUM
    region, woven into the score stream with a lag so the in-order PE queue
    never blocks on late operands.
  - normalize per 512-column chunk as soon as its accumulation stops (copy
    denom, reciprocal_approx_fast, multiply, DMA out) so the tail after the
    last exp is ~2.5us.
"""

import numpy as np
from contextlib import ExitStack

import concourse.bass as bass
import concourse.mybir as mybir
import concourse.tile as tile
from concourse import bacc
from concourse.bass_utils import run_bass_kernel_spmd

E = 768  # n_embd
H = 64  # head size
S = 2048  # sequence length
B = 8  # batch == n_cores
EC = E // 128  # e chunks
TB = S // 128  # t blocks
NP = TB // 2  # t-block pairs
INV_SQRT_C = float(1.0 / np.sqrt(np.float32(E)))
WQK = EC * 128  # 768 packed weight cols per q/k tensor
WV = EC * H  # 384 packed weight cols for v
CW = 512  # q/k DMA+projection chunk width
NC_ = S // CW  # 8 chunks

F16 = mybir.dt.float16
F32 = mybir.dt.float32

_CACHE = {}


def build_program():
    nc = bacc.Bacc(
        "TRN2",
        target_bir_lowering=False,
        debug=False,
        enable_asserts=False,
        num_devices=B,
    )

    # chunk-major host layouts: per-partition rows are >=3KB contiguous
    qT_d = nc.dram_tensor("qTc", [NC_ * 128, EC * CW], F16, kind="ExternalInput")
    kT_d = nc.dram_tensor("kTc", [NC_ * 128, EC * CW], F16, kind="ExternalInput")
    vT_d = nc.dram_tensor("vTc", [2 * 128, EC * 1024], F16, kind="ExternalInput")
    w_d = nc.dram_tensor("wpack", [128, 2 * WQK + WV + H], F16, kind="ExternalInput")
    b_d = nc.dram_tensor("bpack", [128, 4], F32, kind="ExternalInput")
    outT_d = nc.dram_tensor("outT", [H, S], F32, kind="ExternalOutput")

    with tile.TileContext(nc) as tc, ExitStack() as ctx:
        const = ctx.enter_context(tc.tile_pool(name="const", bufs=1))
        xin = ctx.enter_context(tc.tile_pool(name="xin", bufs=1))
        acts = ctx.enter_context(tc.tile_pool(name="acts", bufs=1))

        w_t = const.tile([128, 2 * WQK + WV + H], F16, tag="w")
        b_t = const.tile([128, 4], F32, tag="b")
        warm = const.tile([128, 8], F32, tag="warm")
        wq_t = w_t[:, 0:WQK]
        wk_t = w_t[:, WQK : 2 * WQK]
        wv_t = w_t[:, 2 * WQK : 2 * WQK + WV]
        id_t = w_t[0:H, 2 * WQK + WV : 2 * WQK + WV + H]
        bq_t = b_t[:, 0:1]
        bk_t = b_t[:, 1:2]
        bv_t = b_t[0:H, 2:3]

        q_in = xin.tile([128, S * EC], F16, tag="q_in")
        k_in = xin.tile([128, S * EC], F16, tag="k_in")
        v_in = xin.tile([128, 2 * EC * 1024], F16, tag="v_in")

        qiT = acts.tile([128, S], F16, tag="qiT")
        kiT = acts.tile([128, S], F16, tag="kiT")
        viT = acts.tile([H, S], F16, tag="viT")
        vaug = acts.tile([128, TB * 128], F16, tag="vaug")
        att = acts.tile([128, NP * 4096], F16, tag="att")
        dsb = acts.tile([H, S], F32, tag="dsb")
        rec = acts.tile([H, S], F32, tag="rec")
        out_sb = acts.tile([H, S], F32, tag="out_sb")

        def qk_chunk(x_t, c):
            # [128, EC, CW] view of s-columns [CW*c, CW*(c+1))
            return x_t[:, c * EC * CW : (c + 1) * EC * CW].rearrange(
                "p (e w) -> p e w", w=CW
            )

        def v_chunk(h):
            return v_in[:, h * EC * 1024 : (h + 1) * EC * 1024].rearrange(
                "p (e w) -> p e w", w=1024
            )

        def load_qk(eng, x_t, x_d, c):
            eng.dma_start(
                x_t[:, c * EC * CW : (c + 1) * EC * CW],
                x_d[c * 128 : (c + 1) * 128, :],
            )

        def load_v(eng, h):
            eng.dma_start(
                v_in[:, h * EC * 1024 : (h + 1) * EC * 1024],
                vT_d[h * 128 : (h + 1) * 128, :],
            )

        # ---- Scalar HWDGE queue: the critical gate, then exp-table warm ----
        nc.vector.memset(warm[:], 0.0)
        nc.scalar.dma_start(w_t[:], w_d[:])
        load_qk(nc.scalar, q_in, qT_d, 0)
        load_qk(nc.scalar, q_in, qT_d, 1)
        nc.scalar.activation(
            warm[:], warm[:], mybir.ActivationFunctionType.Exp, scale=1.0
        )

        # ---- Sync HWDGE queue: everything else, in need-time order ----
        nc.sync.dma_start(b_t[:], b_d[:])
        load_qk(nc.sync, k_in, kT_d, 0)
        load_qk(nc.sync, k_in, kT_d, 1)
        load_v(nc.sync, 0)
        load_qk(nc.sync, k_in, kT_d, 2)
        load_qk(nc.sync, k_in, kT_d, 3)

        nc.vector.memset(vaug[:], 1.0)

        with tc.tile_pool(name="ps", bufs=2, space="PSUM") as ps, tc.tile_pool(
            name="op", bufs=1, space="PSUM"
        ) as op:
            po = op.tile([128, S], F32, tag="po")
            out_emitted = [0, 0, 0, 0]

            # ---- PE HAM warm-up on the weight tile while inputs stream ----
            for i in range(6):
                pw = ps.tile([128, 1024], F32, tag="ps")
                nc.tensor.matmul(
                    pw[:, 0:512],
                    lhsT=w_t[:, (i % 6) * 128 : (i % 6) * 128 + 128],
                    rhs=w_t[:, 0:512],
                    start=True,
                    stop=True,
                )

            def proj_chunk(x_t, w_sl, b_sl, dst, c):
                # CW s-columns of a q/k projection (dup'd weights)
                pj = ps.tile([128, 1024], F32, tag="ps")
                xc = qk_chunk(x_t, c)
                for e in range(EC):
                    nc.tensor.matmul(
                        pj[:, 0:CW],
                        lhsT=w_sl[:, e * 128 : (e + 1) * 128],
                        rhs=xc[:, e, :],
                        start=(e == 0),
                        stop=(e == EC - 1),
                    )
                nc.vector.tensor_scalar_add(
                    dst[:, c * CW : (c + 1) * CW], pj[:, 0:CW], b_sl
                )

            def proj_v(h):
                # 1024 s-columns of the v projection (single-width weights)
                pj = ps.tile([128, 1024], F32, tag="ps")
                xc = v_chunk(h)
                for e in range(EC):
                    for n in range(2):
                        nc.tensor.matmul(
                            pj[0:H, n * 512 : (n + 1) * 512],
                            lhsT=wv_t[:, e * H : (e + 1) * H],
                            rhs=xc[:, e, n * 512 : (n + 1) * 512],
                            start=(e == 0),
                            stop=(e == EC - 1),
                        )
                nc.vector.tensor_scalar_add(
                    viT[:, h * 1024 : (h + 1) * 1024], pj[0:H, :], bv_t
                )

            def tr_v(h):
                # viT [64, 8x128] -> vi [128, 8, 64] via PE transpose-mode,
                # then DVE-copy into the [vi | ones] vaug blocks
                tr = ps.tile([128, 512], F16, tag="ps")
                for i in range(8):
                    tb = h * 8 + i
                    nc.tensor.transpose(
                        tr[:, i * H : (i + 1) * H],
                        viT[:, tb * 128 : (tb + 1) * 128],
                        id_t,
                    )
                dst_ap = vaug[:, h * 1024 : (h + 1) * 1024].rearrange(
                    "p (t c) -> p t c", c=128
                )[:, :, 0:H]
                nc.vector.tensor_copy(
                    dst_ap, tr[:].rearrange("p (t k) -> p t k", k=H)
                )

            def unit(p, j):
                # scores + exp for t-blocks (2p, 2p+1) x q-cols [512j, 512j+512)
                u = ps.tile([128, 1024], F32, tag="ps")
                tbE, tbO = 2 * p, 2 * p + 1
                nc.tensor.matmul(
                    u[:, 0:512],
                    lhsT=kiT[0:H, tbE * 128 : (tbE + 1) * 128],
                    rhs=qiT[0:H, j * 512 : (j + 1) * 512],
                    start=True,
                    stop=True,
                )
                nc.tensor.matmul(
                    u[:, 512:1024],
                    lhsT=kiT[H:128, tbO * 128 : (tbO + 1) * 128],
                    rhs=qiT[H:128, j * 512 : (j + 1) * 512],
                    start=True,
                    stop=True,
                )
                nc.scalar.activation(
                    att[:, p * 4096 + j * 1024 : p * 4096 + (j + 1) * 1024],
                    u[:],
                    mybir.ActivationFunctionType.Exp,
                    scale=INV_SQRT_C,
                )

            def outs(tbs, js):
                for j in js:
                    for tb in tbs:
                        base = (tb // 2) * 4096 + j * 1024 + (tb % 2) * 512
                        nc.tensor.matmul(
                            po[:, j * 512 : (j + 1) * 512],
                            lhsT=vaug[:, tb * 128 : (tb + 1) * 128],
                            rhs=att[:, base : base + 512],
                            start=(tb == 0),
                            stop=(tb == TB - 1),
                        )
                        out_emitted[j] += 1
                    if out_emitted[j] == TB:
                        norm(j)

            def norm(j):
                jc = slice(j * 512, (j + 1) * 512)
                nc.vector.tensor_copy(dsb[:, jc], po[H:128, jc])
                nc.vector.reciprocal_approx_fast(rec[:, jc], dsb[:, jc])
                nc.vector.tensor_tensor(
                    out_sb[:, jc], po[0:H, jc], rec[:, jc], op=mybir.AluOpType.mult
                )
                nc.sync.dma_start(outT_d[:, jc], out_sb[:, jc])

            # ---- PE stream, woven so the in-order queue never blocks ----
            proj_chunk(q_in, wq_t, bq_t, qiT, 0)
            proj_chunk(k_in, wk_t, bk_t, kiT, 0)
            unit(0, 0)
            unit(1, 0)
            proj_chunk(q_in, wq_t, bq_t, qiT, 1)
            unit(0, 1)
            unit(1, 1)

            proj_chunk(k_in, wk_t, bk_t, kiT, 1)
            unit(2, 0)
            unit(3, 0)
            unit(2, 1)
            unit(3, 1)
            proj_v(0)

            load_v(nc.sync, 1)

            proj_chunk(k_in, wk_t, bk_t, kiT, 2)
            unit(4, 0)
            unit(5, 0)
            tr_v(0)
            unit(4, 1)
            unit(5, 1)
            outs(range(0, 4), (0, 1))

            load_qk(nc.sync, q_in, qT_d, 2)
            load_qk(nc.sync, q_in, qT_d, 3)

            proj_chunk(k_in, wk_t, bk_t, kiT, 3)
            proj_v(1)
            unit(6, 0)
            unit(7, 0)
            tr_v(1)
            unit(6, 1)
            unit(7, 1)
            outs(range(4, 8), (0, 1))

            proj_chunk(q_in, wq_t, bq_t, qiT, 2)
            unit(0, 2)
            unit(1, 2)
            outs(range(8, 12), (0, 1))
            unit(2, 2)
            unit(3, 2)
            outs(range(12, 16), (0, 1))
            unit(4, 2)
            unit(5, 2)
            outs(range(0, 4), (2,))
            unit(6, 2)
            unit(7, 2)
            outs(range(4, 8), (2,))

            proj_chunk(q_in, wq_t, bq_t, qiT, 3)
            unit(0, 3)
            unit(1, 3)
            outs(range(8, 12), (2,))
            unit(2, 3)
            unit(3, 3)
            outs(range(12, 16), (2,))
            unit(4, 3)
            unit(5, 3)
            outs(range(0, 8), (3,))
            unit(6, 3)
            unit(7, 3)
            outs(range(8, 16), (3,))

    nc.compile()
    return nc


def _prep_inputs(q, k, v, Wq, bq, Wk, bk, Wv, bv):
    """Host-side layout prep: per-batch transpose + fp16 cast + re-chunk."""
    wq2 = np.concatenate([Wq, Wq], axis=1).astype(np.float16)  # [768, 128]
    wk2 = np.concatenate([Wk, Wk], axis=1).astype(np.float16)
    wv1 = np.asarray(Wv, dtype=np.float16)  # [768, 64]
    idp = np.zeros((128, H), dtype=np.float16)
    idp[0:H] = np.eye(H, dtype=np.float16)
    wpack = np.ascontiguousarray(
        np.concatenate(
            [
                wq2.reshape(EC, 128, 128).transpose(1, 0, 2).reshape(128, WQK),
                wk2.reshape(EC, 128, 128).transpose(1, 0, 2).reshape(128, WQK),
                wv1.reshape(EC, 128, H).transpose(1, 0, 2).reshape(128, WV),
                idp,
            ],
            axis=1,
        )
    )
    bpack = np.zeros((128, 4), dtype=np.float32)
    bpack[:, 0] = np.tile(np.asarray(bq, np.float32), 2)
    bpack[:, 1] = np.tile(np.asarray(bk, np.float32), 2)
    bpack[:, 2] = np.tile(np.asarray(bv, np.float32), 2)

    def chunk(xT, w):
        # [768, 2048] -> [(S/w)*128, 6*w]: s-chunk-major, e-chunks packed
        return np.ascontiguousarray(
            xT.reshape(EC, 128, S // w, w)
            .transpose(2, 1, 0, 3)
            .reshape((S // w) * 128, EC * w)
        )

    in_maps = []
    for i in range(B):
        qT = np.asarray(q[i], np.float16).T
        kT = np.asarray(k[i], np.float16).T
        vT = np.asarray(v[i], np.float16).T
        m = {
            "qTc": chunk(qT, CW),
            "kTc": chunk(kT, CW),
            "vTc": chunk(vT, 1024),
            "wpack": wpack,
            "bpack": bpack,
        }
        in_maps.append(m)
    return in_maps


def run(trace=False, **inputs):
    """Build (cached), run on 8 cores, gather. Returns (out, BassKernelResults)."""
    if "nc" not in _CACHE:
        _CACHE["nc"] = build_program()
    nc = _CACHE["nc"]
    in_maps = _prep_inputs(**{k2: np.asarray(v2) for k2, v2 in inputs.items()})
    res = run_bass_kernel_spmd(nc, in_maps, list(range(B)), trace=trace)
    out = np.stack([np.ascontiguousarray(res.results[i]["outT"].T) for i in range(B)])
    return out.astype(np.float32), res


def kernel(**inputs) -> np.ndarray:
    out, _ = run(trace=False, **inputs)
    return out
